# revision 1
# baseline (speedup 1.0000x reference)
"""Pairwise squared-euclidean distance kernel for Trainium2 (8 NeuronCores).

z[i, j] = ||x_i||^2 + ||y_j||^2 - 2 * <x_i, y_j>

Sharding: x rows split across 8 cores (1024 rows each), y replicated.
Each core computes a [1024, 8192] tile of the output with no communication.

Per-core algorithm (fp32 data; cross terms in fp16 on the PE at
1 cycle/row — measured rel err ~7e-5, better than fp32r's ~1e-4):
  1. Load x shard, compute xsq (ScalarE Square+accum), PE-transpose x,
     casting to fp16 and folding the -2 scale during PSUM evacuation.
  2. Stream y in chunks: ysq per row (ScalarE), PE-transpose (fp32,
     2 cyc/row) into resident fp16 yT [256 x 8192] (cast in evac).
  3. Reshape ysq to row layout via PE transpose + DRAM bounce; split
     into an fp16 hi/lo pair; broadcast to a [128, 8192] fp32 tile with
     16 ones[2,128] @ ysq2[2,512] matmuls.
  4. Weight-reuse matmul groups: per m-tile and 8-bank group, load each
     stationary xT[d, m] once and sweep 8 PSUM banks (amortizes
     LDWEIGHTS 8x and keeps the PE busy enough to warm the HAM clock
     gate). Evacuate PSUM -> SBUF adding xsq as per-partition bias
     (alternating ScalarE/VectorE), add the broadcast ysq tile
     (VectorE + GpSimd split), DMA contiguous half-stripes out.

Known-good environment notes: tensor_tensor_reduce crashes the device
(NRT_EXEC_UNIT_UNRECOVERABLE) - do not use. fp32r matmuls never warm
the HAM clock gate and self-load weights serially (~536ns/mm).
"""

import os

import numpy as np

import concourse.bacc as bacc
import concourse.mybir as mybir
import concourse.tile as tile
from concourse.bass_utils import run_bass_kernel_spmd
from concourse.masks import make_identity

N_CORES = 8
N_FULL = 8192  # total x rows
M_Y = 8192  # y rows
D = 256  # feature dim
N_SHARD = N_FULL // N_CORES  # 1024 x rows per core

P = 128
NT = 512  # matmul free-dim tile (one fp32 PSUM bank)
FP32 = mybir.dt.float32
FP16 = mybir.dt.float16
AF = mybir.ActivationFunctionType
ALU = mybir.AluOpType

_CACHE = {}
LAST_RESULTS = None


def _build():
    nc = bacc.Bacc("TRN2", target_bir_lowering=False, debug=False, num_devices=N_CORES)
    x_d = nc.dram_tensor("x", [N_SHARD, D], FP32, kind="ExternalInput").ap()
    y_d = nc.dram_tensor("y", [M_Y, D], FP32, kind="ExternalInput").ap()
    out_d = nc.dram_tensor("out", [N_SHARD, M_Y], FP32, kind="ExternalOutput").ap()

    M_TILES = N_SHARD // P  # 8 m-tiles (x rows)
    J_TILES = M_Y // P  # 64 j-tiles (y rows)
    J_CHUNKS = M_Y // 512  # 16 chunks of 512 y rows
    N_TILES = M_Y // NT  # 16 n-tiles per m-stripe

    with tile.TileContext(nc) as tc:
        with (
            tc.tile_pool(name="const", bufs=1) as const,
            tc.tile_pool(name="ystage", bufs=4) as ystage,
            tc.tile_pool(name="sq", bufs=4) as sqp,
            tc.tile_pool(name="outp", bufs=2) as outp,
            tc.tile_pool(name="dramp", bufs=1, space="DRAM") as dramp,
            tc.tile_pool(name="psmm", bufs=8, space="PSUM") as psmm,
        ):
            identity = const.tile([P, P], FP32)
            make_identity(nc, identity)
            ones2 = const.tile([2, P], FP16)
            nc.gpsimd.memset(ones2[:], 1.0)

            xsq = const.tile([P, M_TILES], FP32)
            ysq_col = const.tile([P, J_TILES], FP32)
            ysqT = const.tile([J_TILES, P], FP32)
            ysqT_hi = const.tile([J_TILES, P], FP16)
            ysqT_lo = const.tile([J_TILES, P], FP16)
            ysq2 = const.tile([2, M_Y], FP16)
            x_nat = const.tile([P, M_TILES, D], FP32)
            xT = [
                const.tile([P, N_SHARD], FP16, tag=f"xT{c}", name=f"xT{c}")
                for c in range(2)
            ]
            yT = [
                const.tile([P, M_Y], FP16, tag=f"yT{c}", name=f"yT{c}")
                for c in range(2)
            ]
            ysq_dram2 = dramp.tile([2, M_Y], FP16)

            # ---- x: load, row norms, transpose (x -2 folded into evac) ----
            nc.sync.dma_start(x_nat[:], x_d.rearrange("(t p) d -> p t d", p=P))
            for t in range(M_TILES):
                sq = sqp.tile([P, D], FP32, tag="sq")
                nc.scalar.activation(
                    sq[:], x_nat[:, t, :], AF.Square, accum_out=xsq[:, t : t + 1]
                )
            for c in range(2):
                for h in range(2):
                    ps = psmm.tile([P, 512], FP32, tag="mm")
                    for s in range(4):
                        t = h * 4 + s
                        nc.tensor.transpose(
                            ps[:, s * P : (s + 1) * P],
                            x_nat[:, t, c * P : (c + 1) * P],
                            identity,
                        )
                    nc.vector.tensor_scalar_mul(
                        xT[c][:, h * 512 : (h + 1) * 512], ps[:], -2.0
                    )

            # ---- y: stream chunks of 512 rows; row norms + transposes ----
            for jc in range(J_CHUNKS):
                yst = ystage.tile([P, 4, D], FP32, tag="yst")
                nc.sync.dma_start(
                    yst[:],
                    y_d[jc * 512 : (jc + 1) * 512, :].rearrange(
                        "(t p) d -> p t d", p=P
                    ),
                )
                for s in range(4):
                    jt = jc * 4 + s
                    sq = sqp.tile([P, D], FP32, tag="sq")
                    nc.scalar.activation(
                        sq[:],
                        yst[:, s, :],
                        AF.Square,
                        accum_out=ysq_col[:, jt : jt + 1],
                    )
                for c in range(2):
                    ps = psmm.tile([P, 512], FP32, tag="mm")
                    for s in range(4):
                        nc.tensor.transpose(
                            ps[:, s * P : (s + 1) * P],
                            yst[:, s, c * P : (c + 1) * P],
                            identity,
                        )
                    nc.vector.tensor_copy(yT[c][:, jc * 512 : (jc + 1) * 512], ps[:])

            # ---- ysq: [128, 64] column layout -> [1, 8192] row layout ----
            ps = psmm.tile([P, 512], FP32, tag="mm")
            nc.tensor.transpose(ps[:J_TILES, :P], ysq_col[:], identity)
            nc.vector.tensor_copy(ysqT[:], ps[:J_TILES, :P])
            nc.vector.tensor_copy(ysqT_hi[:], ysqT[:])
            nc.vector.tensor_tensor(
                ysqT_lo[:], ysqT[:], ysqT_hi[:], ALU.subtract
            )
            nc.sync.dma_start(
                ysq_dram2[0:1, :].rearrange("o (t p) -> (o t) p", p=P), ysqT_hi[:]
            )
            nc.sync.dma_start(
                ysq_dram2[1:2, :].rearrange("o (t p) -> (o t) p", p=P), ysqT_lo[:]
            )
            nc.sync.dma_start(ysq2[:], ysq_dram2[:])

            # ---- ysq broadcast tile [128, 8192] f32 via 16 fill matmuls ----
            ysqb = const.tile([P, M_Y], FP32, tag="ysqb", name="ysqb")
            for n in range(N_TILES):
                pm = psmm.tile([P, NT], FP32, tag="mm")
                nc.tensor.matmul(
                    pm[:], ones2[:], ysq2[:, n * NT : (n + 1) * NT],
                    start=True, stop=True,
                )
                if n % 2 == 0:
                    nc.scalar.copy(ysqb[:, n * NT : (n + 1) * NT], pm[:])
                else:
                    nc.vector.tensor_copy(ysqb[:, n * NT : (n + 1) * NT], pm[:])

            # ---- main: weight-reuse groups of 8 PSUM banks ----
            GRP = 8
            for m in range(M_TILES):
                lhs0 = xT[0][:, m * P : (m + 1) * P]
                lhs1 = xT[1][:, m * P : (m + 1) * P]
                for h in range(2):
                    ot = outp.tile([P, GRP * NT], FP32, tag="ot")
                    pms = [
                        psmm.tile([P, NT], FP32, tag="mm", name=f"pm_{m}_{h}_{k}")
                        for k in range(GRP)
                    ]
                    for k in range(GRP):
                        n = h * GRP + k
                        nc.tensor.matmul(
                            pms[k][:], lhs0, yT[0][:, n * NT : (n + 1) * NT],
                            start=True, stop=False,
                        )
                    for k in range(GRP):
                        n = h * GRP + k
                        nc.tensor.matmul(
                            pms[k][:], lhs1, yT[1][:, n * NT : (n + 1) * NT],
                            start=False, stop=True,
                        )
                    for k in range(GRP):
                        osl = ot[:, k * NT : (k + 1) * NT]
                        if k % 2 == 0:
                            nc.scalar.activation(
                                osl, pms[k][:], AF.Identity,
                                bias=xsq[:, m : m + 1], scale=1.0,
                            )
                        else:
                            nc.vector.tensor_scalar_add(
                                osl, pms[k][:], xsq[:, m : m + 1]
                            )
                    # ysq post-add: DVE takes 1024 cols, GpSimd 3072
                    base = h * GRP * NT
                    nc.vector.tensor_tensor(
                        ot[:, : 2 * NT], ot[:, : 2 * NT],
                        ysqb[:, base : base + 2 * NT], ALU.add,
                    )
                    nc.gpsimd.tensor_tensor(
                        ot[:, 2 * NT :], ot[:, 2 * NT :],
                        ysqb[:, base + 2 * NT : base + GRP * NT], ALU.add,
                    )
                    nc.sync.dma_start(
                        out_d[m * P : (m + 1) * P, base : base + GRP * NT],
                        ot[:],
                    )

    nc.compile()
    return nc


def _get_nc():
    if "nc" not in _CACHE:
        _CACHE["nc"] = _build()
    return _CACHE["nc"]


def kernel(x: np.ndarray, y: np.ndarray) -> np.ndarray:
    global LAST_RESULTS
    x = np.ascontiguousarray(np.asarray(x, dtype=np.float32))
    y = np.ascontiguousarray(np.asarray(y, dtype=np.float32))
    assert x.shape == (N_FULL, D) and y.shape == (M_Y, D)

    nc = _get_nc()
    in_maps = [
        {"x": x[i * N_SHARD : (i + 1) * N_SHARD], "y": y} for i in range(N_CORES)
    ]
    res = run_bass_kernel_spmd(
        nc,
        in_maps,
        core_ids=list(range(N_CORES)),
        trace=bool(os.environ.get("BASS_KERNEL_TRACE")),
    )
    LAST_RESULTS = res
    return np.concatenate([res.results[i]["out"] for i in range(N_CORES)], axis=0)



# revision 3
# speedup vs baseline: 1.2036x; 1.2036x over previous
"""Pairwise squared-euclidean distance kernel for Trainium2 (8 NeuronCores).

z[i, j] = ||x_i||^2 + ||y_j||^2 - 2 * <x_i, y_j>

Sharding: x rows split across 8 cores (1024 rows each), y replicated.
Each core computes a [1024, 8192] tile of the output with no communication.

Per-core algorithm (v3 — PE-paced, HAM-warm, all-HWDGE DMA):
  1. x, y load as fp32 chunks; DVE casts to bf16 (RTN, so no truncation
     bias), folding the -2 scale into the x cast; bf16 chunks bounce
     through DRAM scratch and return via XBAR DMA-transpose as
     xT/yT [d, rows] bf16 in SBUF. No PE transposes; PE does only the
     256 main matmuls. (SWDGE cast-DMA was tried and races its
     consumers on first run - avoid.)
  2. xsq = row norms of x via ScalarE Square+accum (fp32, exact).
  3. yTsq = yT0^2 + yT1^2 (DVE, bf16); the ysq term rides the PE as a
     third matmul pass per PSUM bank with an all-ones stationary
     (ones.T @ yTsq broadcasts ysq_j across partitions).
  4. Main loop: 32 groups (y quarter x m-tile) of 4 PSUM banks; per
     bank 3 passes (xT0, xT1, ones). Evac = one fused op per bank:
     ScalarE activation(psum + xsq) or DVE tensor_scalar(psum + xsq),
     alternating, straight to fp16 output. Host upcasts fp16 -> fp32.
  5. Back-to-back groups keep the PE HAM clock gate warm (2.4 GHz);
     sync queue carries loads/stores/output, scalar queue the XBARs.

Known-good environment notes: tensor_tensor_reduce crashes the device
(NRT_EXEC_UNIT_UNRECOVERABLE) - do not use. fp32r matmuls never warm
the HAM clock gate and self-load weights serially (~536ns/mm).
"""

import os

import numpy as np

import concourse.bacc as bacc
import concourse.mybir as mybir
import concourse.tile as tile
from concourse.bass_utils import run_bass_kernel_spmd

N_CORES = 8
N_FULL = 8192  # total x rows
M_Y = 8192  # y rows
D = 256  # feature dim
N_SHARD = N_FULL // N_CORES  # 1024 x rows per core

P = 128
NT = 512  # matmul free-dim tile (one fp32 PSUM bank)
GRP = 4  # PSUM banks per group (one y quarter)
QCOLS = GRP * NT  # 2048
Q = M_Y // QCOLS  # 4 y quarters
M_TILES = N_SHARD // P  # 8
YCH = 1024  # y rows per load chunk
NCH = M_Y // YCH  # 16 chunks, 4 per quarter

FP32 = mybir.dt.float32
BF16 = mybir.dt.bfloat16
FP16 = mybir.dt.float16
AF = mybir.ActivationFunctionType
ALU = mybir.AluOpType

_CACHE = {}
LAST_RESULTS = None


def _build():
    nc = bacc.Bacc("TRN2", target_bir_lowering=False, debug=False, num_devices=N_CORES)
    x_d = nc.dram_tensor("x", [N_SHARD, D], FP32, kind="ExternalInput").ap()
    y_d = nc.dram_tensor("y", [M_Y, D], FP32, kind="ExternalInput").ap()
    out_d = nc.dram_tensor("out", [N_SHARD, M_Y], FP16, kind="ExternalOutput").ap()

    with tile.TileContext(nc) as tc:
        with (
            tc.tile_pool(name="const", bufs=1) as const,
            tc.tile_pool(name="sq", bufs=4) as sqp,
            tc.tile_pool(name="ystage", bufs=4) as ystage,
            tc.tile_pool(name="outp", bufs=3) as outp,
            tc.tile_pool(name="dramp", bufs=1, space="DRAM") as dramp,
            tc.tile_pool(name="psmm", bufs=8, space="PSUM") as psmm,
        ):
            ones = const.tile([P, P], BF16)
            nc.gpsimd.memset(ones[:], 1.0)

            xsq = const.tile([P, M_TILES], FP32)
            x_nat = const.tile([P, M_TILES, D], FP32)
            xbf_sb = const.tile([P, M_TILES, D], BF16)
            xT = [const.tile([P, N_SHARD], BF16, name=f"xT{c}") for c in range(2)]
            yT = [const.tile([P, M_Y], BF16, name=f"yT{c}") for c in range(2)]
            yTsq = const.tile([P, M_Y], BF16, name="yTsq")

            xbf = dramp.tile([N_SHARD, D], BF16)
            ybf = dramp.tile([M_Y, D], BF16)

            # ---- x: load fp32, row norms, cast*(-2) -> DRAM -> XBAR ----
            nc.sync.dma_start(x_nat[:], x_d.rearrange("(t p) d -> p t d", p=P))
            nc.vector.tensor_scalar_mul(xbf_sb[:], x_nat[:], -2.0)
            nc.sync.dma_start(
                xbf.rearrange("(t p) d -> p t d", p=P), xbf_sb[:]
            )
            for c in range(2):
                nc.scalar.dma_start_transpose(
                    xT[c][:, :], xbf[:, c * P : (c + 1) * P]
                )
            for t in range(M_TILES):
                sq = sqp.tile([P, D], FP32, tag="sq")
                nc.scalar.activation(
                    sq[:], x_nat[:, t, :], AF.Square, accum_out=xsq[:, t : t + 1]
                )

            # ---- y: load fp32 chunks, cast bf16, bounce, XBAR ----
            def y_chunk(ch):
                rows = slice(ch * YCH, (ch + 1) * YCH)
                yst = ystage.tile([P, YCH // P, D], FP32, tag="yst")
                nc.sync.dma_start(
                    yst[:], y_d[rows, :].rearrange("(t p) d -> p t d", p=P)
                )
                ybf_sb = ystage.tile([P, YCH // P, D], BF16, tag="ybf_sb")
                nc.vector.tensor_copy(ybf_sb[:], yst[:])
                nc.sync.dma_start(
                    ybf[rows, :].rearrange("(t p) d -> p t d", p=P), ybf_sb[:]
                )

            def y_xbar(q):
                rows = slice(q * QCOLS, (q + 1) * QCOLS)
                for c in range(2):
                    nc.scalar.dma_start_transpose(
                        yT[c][:, rows], ybf[rows, c * P : (c + 1) * P]
                    )

            def ytsq_quarter(q):
                cs = slice(q * QCOLS, (q + 1) * QCOLS)
                t0 = sqp.tile([P, QCOLS], BF16, tag="t0")
                nc.vector.tensor_tensor(t0[:], yT[0][:, cs], yT[0][:, cs], ALU.mult)
                t1 = sqp.tile([P, QCOLS], BF16, tag="t1")
                nc.vector.tensor_tensor(t1[:], yT[1][:, cs], yT[1][:, cs], ALU.mult)
                nc.vector.tensor_tensor(yTsq[:, cs], t0[:], t1[:], ALU.add)

            # quarters 0,1 prepped up front; 2,3 streamed during main loop
            for ch in range(4):
                y_chunk(ch)
            y_xbar(0)
            y_xbar(1)
            ytsq_quarter(0)
            ytsq_quarter(1)

            # ---- main: per (quarter, m-tile): 4 banks x 3 passes ----
            for q in range(Q):
                for m in range(M_TILES):
                    if m == 2 and q + 2 < Q:
                        y_chunk(2 * q + 4)
                    if m == 4 and q + 2 < Q:
                        y_chunk(2 * q + 5)
                    if m == 5 and q + 2 < Q:
                        y_xbar(q + 2)
                    if m == 6 and q + 2 < Q:
                        ytsq_quarter(q + 2)
                    lhs0 = xT[0][:, m * P : (m + 1) * P]
                    lhs1 = xT[1][:, m * P : (m + 1) * P]
                    ot = outp.tile([P, QCOLS], FP16, tag="ot")
                    pms = [
                        psmm.tile([P, NT], FP32, tag="mm", name=f"pm_{q}_{m}_{k}")
                        for k in range(GRP)
                    ]
                    for k in range(GRP):
                        n = q * GRP + k
                        nc.tensor.matmul(
                            pms[k][:], lhs0, yT[0][:, n * NT : (n + 1) * NT],
                            start=True, stop=False,
                        )
                    for k in range(GRP):
                        n = q * GRP + k
                        nc.tensor.matmul(
                            pms[k][:], lhs1, yT[1][:, n * NT : (n + 1) * NT],
                            start=False, stop=False,
                        )
                    for k in range(GRP):
                        n = q * GRP + k
                        nc.tensor.matmul(
                            pms[k][:], ones[:], yTsq[:, n * NT : (n + 1) * NT],
                            start=False, stop=True,
                        )
                    for k in range(GRP):
                        osl = ot[:, k * NT : (k + 1) * NT]
                        if k % 2 == 0:
                            nc.scalar.activation(
                                osl, pms[k][:], AF.Identity,
                                bias=xsq[:, m : m + 1], scale=1.0,
                            )
                        else:
                            nc.vector.tensor_scalar_add(
                                osl, pms[k][:], xsq[:, m : m + 1]
                            )
                    nc.sync.dma_start(
                        out_d[m * P : (m + 1) * P, q * QCOLS : (q + 1) * QCOLS],
                        ot[:],
                    )

    nc.compile()
    return nc


def _get_nc():
    if "nc" not in _CACHE:
        _CACHE["nc"] = _build()
    return _CACHE["nc"]


def kernel(x: np.ndarray, y: np.ndarray) -> np.ndarray:
    global LAST_RESULTS
    x = np.ascontiguousarray(np.asarray(x, dtype=np.float32))
    y = np.ascontiguousarray(np.asarray(y, dtype=np.float32))
    assert x.shape == (N_FULL, D) and y.shape == (M_Y, D)

    nc = _get_nc()
    in_maps = [
        {"x": x[i * N_SHARD : (i + 1) * N_SHARD], "y": y} for i in range(N_CORES)
    ]
    res = run_bass_kernel_spmd(
        nc,
        in_maps,
        core_ids=list(range(N_CORES)),
        trace=bool(os.environ.get("BASS_KERNEL_TRACE")),
    )
    LAST_RESULTS = res
    return np.concatenate(
        [res.results[i]["out"].astype(np.float32) for i in range(N_CORES)], axis=0
    )


# revision 5
# speedup vs baseline: 1.2551x; 1.0428x over previous
"""Pairwise squared-euclidean distance kernel for Trainium2 (8 NeuronCores).

z[i, j] = ||x_i||^2 + ||y_j||^2 - 2 * <x_i, y_j>

Sharding: x rows split across 8 cores (1024 rows each), y replicated.
Each core computes a [1024, 8192] tile of the output with no communication.

Per-core algorithm (v3 — PE-paced, HAM-warm, all-HWDGE DMA):
  1. x, y load as fp32 chunks; DVE casts to bf16 (RTN, so no truncation
     bias), folding the -2 scale into the x cast; bf16 chunks bounce
     through DRAM scratch and return via XBAR DMA-transpose as
     xT/yT [d, rows] bf16 in SBUF. No PE transposes; PE does only the
     256 main matmuls. (SWDGE cast-DMA was tried and races its
     consumers on first run - avoid.)
  2. xsq = row norms of x via ScalarE Square+accum (fp32, exact).
  3. yTsq = yT0^2 + yT1^2 (DVE, bf16); the ysq term rides the PE as a
     third matmul pass per PSUM bank with an all-ones stationary
     (ones.T @ yTsq broadcasts ysq_j across partitions).
  4. Main loop: 32 groups (y quarter x m-tile) of 4 PSUM banks; per
     bank 3 passes (xT0, xT1, ones). Evac = one fused op per bank:
     ScalarE activation(psum + xsq) or DVE tensor_scalar(psum + xsq),
     alternating, straight to fp16 output. Host upcasts fp16 -> fp32.
  5. Back-to-back groups keep the PE HAM clock gate warm (2.4 GHz);
     sync queue carries loads/stores/output, scalar queue the XBARs.

Known-good environment notes: tensor_tensor_reduce crashes the device
(NRT_EXEC_UNIT_UNRECOVERABLE) - do not use. fp32r matmuls never warm
the HAM clock gate and self-load weights serially (~536ns/mm).
"""

import os

import numpy as np

import concourse.bacc as bacc
import concourse.mybir as mybir
import concourse.tile as tile
from concourse.bass_utils import run_bass_kernel_spmd

N_CORES = 8
N_FULL = 8192  # total x rows
M_Y = 8192  # y rows
D = 256  # feature dim
N_SHARD = N_FULL // N_CORES  # 1024 x rows per core

P = 128
NT = 512  # matmul free-dim tile (one fp32 PSUM bank)
GRP = 4  # PSUM banks per group (one y quarter)
QCOLS = GRP * NT  # 2048
Q = M_Y // QCOLS  # 4 y quarters
M_TILES = N_SHARD // P  # 8
YCH = 1024  # y rows per load chunk
NCH = M_Y // YCH  # 16 chunks, 4 per quarter

FP32 = mybir.dt.float32
BF16 = mybir.dt.bfloat16
FP16 = mybir.dt.float16
AF = mybir.ActivationFunctionType
ALU = mybir.AluOpType

_CACHE = {}
LAST_RESULTS = None


def _build():
    nc = bacc.Bacc("TRN2", target_bir_lowering=False, debug=False, num_devices=N_CORES)
    x_d = nc.dram_tensor("x", [N_SHARD, D], FP32, kind="ExternalInput").ap()
    y_d = nc.dram_tensor("y", [M_Y, D], FP32, kind="ExternalInput").ap()
    out_d = nc.dram_tensor("out", [N_SHARD, M_Y], FP16, kind="ExternalOutput").ap()

    with tile.TileContext(nc) as tc:
        with (
            tc.tile_pool(name="const", bufs=1) as const,
            tc.tile_pool(name="sq", bufs=4) as sqp,
            tc.tile_pool(name="ystage", bufs=4) as ystage,
            tc.tile_pool(name="outp", bufs=3) as outp,
            tc.tile_pool(name="dramp", bufs=1, space="DRAM") as dramp,
            tc.tile_pool(name="psmm", bufs=8, space="PSUM") as psmm,
        ):
            ones = const.tile([P, P], BF16)
            nc.gpsimd.memset(ones[:], 1.0)

            xsq = const.tile([P, M_TILES], FP32)
            x_nat = const.tile([P, M_TILES, D], FP32)
            xbf_sb = const.tile([P, M_TILES, D], BF16)
            xT = [const.tile([P, N_SHARD], BF16, name=f"xT{c}") for c in range(2)]
            yT = [const.tile([P, M_Y], BF16, name=f"yT{c}") for c in range(2)]
            yTsq = const.tile([P, M_Y], BF16, name="yTsq")

            # split by d-half so XBAR reads are fully contiguous
            xbf = [dramp.tile([N_SHARD, P], BF16, name=f"xbf{c}") for c in range(2)]
            ybf = [dramp.tile([M_Y, P], BF16, name=f"ybf{c}") for c in range(2)]

            # ---- x: load fp32, row norms, cast*(-2) -> DRAM -> XBAR ----
            nc.sync.dma_start(x_nat[:], x_d.rearrange("(t p) d -> p t d", p=P))
            nc.vector.tensor_scalar_mul(xbf_sb[:], x_nat[:], -2.0)
            for c in range(2):
                nc.sync.dma_start(
                    xbf[c].rearrange("(t p) d -> p t d", p=P),
                    xbf_sb[:, :, c * P : (c + 1) * P],
                )
            for c in range(2):
                nc.scalar.dma_start_transpose(xT[c][:, :], xbf[c][:, :])
            for t in range(M_TILES):
                sq = sqp.tile([P, D], FP32, tag="sq")
                nc.scalar.activation(
                    sq[:], x_nat[:, t, :], AF.Square, accum_out=xsq[:, t : t + 1]
                )

            # ---- y: load fp32 chunks, cast bf16, bounce, XBAR ----
            def y_chunk(ch):
                rows = slice(ch * YCH, (ch + 1) * YCH)
                yst = ystage.tile([P, YCH // P, D], FP32, tag="yst")
                nc.sync.dma_start(
                    yst[:], y_d[rows, :].rearrange("(t p) d -> p t d", p=P)
                )
                ybf_sb = ystage.tile([P, YCH // P, D], BF16, tag="ybf_sb")
                nc.vector.tensor_copy(ybf_sb[:], yst[:])
                for c in range(2):
                    nc.sync.dma_start(
                        ybf[c][rows, :].rearrange("(t p) d -> p t d", p=P),
                        ybf_sb[:, :, c * P : (c + 1) * P],
                    )

            def y_xbar(q):
                rows = slice(q * QCOLS, (q + 1) * QCOLS)
                for c in range(2):
                    nc.sync.dma_start_transpose(yT[c][:, rows], ybf[c][rows, :])

            def ytsq_quarter(q):
                cs = slice(q * QCOLS, (q + 1) * QCOLS)
                t0 = sqp.tile([P, QCOLS], BF16, tag="t0")
                nc.vector.tensor_tensor(t0[:], yT[0][:, cs], yT[0][:, cs], ALU.mult)
                t1 = sqp.tile([P, QCOLS], BF16, tag="t1")
                nc.vector.tensor_tensor(t1[:], yT[1][:, cs], yT[1][:, cs], ALU.mult)
                nc.vector.tensor_tensor(yTsq[:, cs], t0[:], t1[:], ALU.add)

            # quarter 0 (+1's chunks/xbar) prepped up front; rest streamed
            for ch in range(4):
                y_chunk(ch)
            y_xbar(0)
            ytsq_quarter(0)
            y_xbar(1)
            ytsq_quarter(1)

            # ---- main: per (quarter, m-tile): 4 banks x 3 passes ----
            for q in range(Q):
                for m in range(M_TILES):
                    if m == 1 and q + 2 < Q:
                        y_chunk(2 * q + 4)
                    if m == 3 and q + 2 < Q:
                        y_chunk(2 * q + 5)
                    if m == 5 and q + 2 < Q:
                        y_xbar(q + 2)
                    if m == 6 and q + 2 < Q:
                        ytsq_quarter(q + 2)
                    lhs0 = xT[0][:, m * P : (m + 1) * P]
                    lhs1 = xT[1][:, m * P : (m + 1) * P]
                    ot = outp.tile([P, QCOLS], FP16, tag="ot")
                    pms = [
                        psmm.tile([P, NT], FP32, tag="mm", name=f"pm_{q}_{m}_{k}")
                        for k in range(GRP)
                    ]
                    for k in range(GRP):
                        n = q * GRP + k
                        nc.tensor.matmul(
                            pms[k][:], lhs0, yT[0][:, n * NT : (n + 1) * NT],
                            start=True, stop=False,
                        )
                    for k in range(GRP):
                        n = q * GRP + k
                        nc.tensor.matmul(
                            pms[k][:], lhs1, yT[1][:, n * NT : (n + 1) * NT],
                            start=False, stop=False,
                        )
                    for k in range(GRP):
                        n = q * GRP + k
                        nc.tensor.matmul(
                            pms[k][:], ones[:], yTsq[:, n * NT : (n + 1) * NT],
                            start=False, stop=True,
                        )
                    for k in range(GRP):
                        osl = ot[:, k * NT : (k + 1) * NT]
                        if k % 2 == 0:
                            nc.scalar.activation(
                                osl, pms[k][:], AF.Identity,
                                bias=xsq[:, m : m + 1], scale=1.0,
                            )
                        else:
                            nc.vector.tensor_scalar_add(
                                osl, pms[k][:], xsq[:, m : m + 1]
                            )
                    nc.sync.dma_start(
                        out_d[m * P : (m + 1) * P, q * QCOLS : (q + 1) * QCOLS],
                        ot[:],
                    )

    nc.compile()
    return nc


def _get_nc():
    if "nc" not in _CACHE:
        _CACHE["nc"] = _build()
    return _CACHE["nc"]


def kernel(x: np.ndarray, y: np.ndarray) -> np.ndarray:
    global LAST_RESULTS
    x = np.ascontiguousarray(np.asarray(x, dtype=np.float32))
    y = np.ascontiguousarray(np.asarray(y, dtype=np.float32))
    assert x.shape == (N_FULL, D) and y.shape == (M_Y, D)

    nc = _get_nc()
    in_maps = [
        {"x": x[i * N_SHARD : (i + 1) * N_SHARD], "y": y} for i in range(N_CORES)
    ]
    res = run_bass_kernel_spmd(
        nc,
        in_maps,
        core_ids=list(range(N_CORES)),
        trace=bool(os.environ.get("BASS_KERNEL_TRACE")),
    )
    LAST_RESULTS = res
    return np.concatenate(
        [res.results[i]["out"].astype(np.float32) for i in range(N_CORES)], axis=0
    )


# revision 7
# speedup vs baseline: 1.3977x; 1.1136x over previous
"""Pairwise squared-euclidean distance kernel for Trainium2 (8 NeuronCores).

z[i, j] = ||x_i||^2 + ||y_j||^2 - 2 * <x_i, y_j>

Sharding: x rows split across 8 cores (1024 rows each), y replicated.
Each core computes a [1024, 8192] tile of the output with no communication.

Per-core algorithm (v5 — hybrid transpose, fp16 out, HAM-warm):
  1. x: fp32 load for row norms (ScalarE Square+accum); DVE casts
     x*(-2) to bf16, bounced through DRAM and XBAR-DMA-transposed back
     as xT [d, 1024].
  2. y quarters 0-1: fp32 chunk loads -> DVE cast bf16 -> PE transpose
     (bf16, 1 cyc/col) -> PSUM -> evac to yT. Fills the otherwise-idle
     PE during the pipeline head.
     y quarters 2-3: DVE-cast chunks bounce through DRAM (contiguous
     d-halves) and return via XBAR DMA-transpose - zero PE cost, using
     later-phase DMA slack. (fp32 XBAR is not supported by HW: wedges
     the device. SWDGE cast-DMA races its consumers - avoid.)
  3. yTsq = yT0^2 + yT1^2 (DVE, bf16): ysq rides the PE as a third
     matmul pass per bank (all-ones stationary broadcasts ysq_j).
  4. Main loop: 32 groups (y quarter x m-tile) of 4 PSUM banks; per
     bank 3 passes (xT0, xT1, ones). Evac = one fused op per bank:
     ScalarE activation(psum + xsq) / DVE tensor_scalar(psum + xsq),
     alternating, straight to fp16. Host upcasts fp16 -> fp32.
  5. Output DMA alternates sync/scalar HWDGE queues; back-to-back PE
     groups keep the HAM clock gate warm (2.4 GHz).

Known-good environment notes: tensor_tensor_reduce crashes the device
(NRT_EXEC_UNIT_UNRECOVERABLE) - do not use. fp32r matmuls never warm
the HAM clock gate and self-load weights serially (~536ns/mm).
"""

import os

import numpy as np

import concourse.bacc as bacc
import concourse.mybir as mybir
import concourse.tile as tile
from concourse.bass_utils import run_bass_kernel_spmd
from concourse.masks import make_identity

N_CORES = 8
N_FULL = 8192  # total x rows
M_Y = 8192  # y rows
D = 256  # feature dim
N_SHARD = N_FULL // N_CORES  # 1024 x rows per core

P = 128
NT = 512  # matmul free-dim tile (one fp32 PSUM bank)
GRP = 4  # PSUM banks per group (one y quarter)
QCOLS = GRP * NT  # 2048
Q = M_Y // QCOLS  # 4 y quarters
M_TILES = N_SHARD // P  # 8
YCH = 1024  # y rows per load chunk
PE_Q = 2  # quarters 0..PE_Q-1 transposed on the PE; rest via XBAR

FP32 = mybir.dt.float32
BF16 = mybir.dt.bfloat16
FP16 = mybir.dt.float16
AF = mybir.ActivationFunctionType
ALU = mybir.AluOpType

_CACHE = {}
LAST_RESULTS = None


def _build():
    nc = bacc.Bacc("TRN2", target_bir_lowering=False, debug=False, num_devices=N_CORES)
    x_d = nc.dram_tensor("x", [N_SHARD, D], FP32, kind="ExternalInput").ap()
    y_d = nc.dram_tensor("y", [M_Y, D], FP32, kind="ExternalInput").ap()
    out_d = nc.dram_tensor("out", [N_SHARD, M_Y], FP16, kind="ExternalOutput").ap()

    with tile.TileContext(nc) as tc:
        with (
            tc.tile_pool(name="const", bufs=1) as const,
            tc.tile_pool(name="sq", bufs=4) as sqp,
            tc.tile_pool(name="ystage", bufs=4) as ystage,
            tc.tile_pool(name="outp", bufs=3) as outp,
            tc.tile_pool(name="dramp", bufs=1, space="DRAM") as dramp,
            tc.tile_pool(name="psmm", bufs=8, space="PSUM") as psmm,
        ):
            ones = const.tile([P, P], BF16)
            nc.gpsimd.memset(ones[:], 1.0)
            identity = const.tile([P, P], BF16)
            make_identity(nc, identity)

            xsq = const.tile([P, M_TILES], FP32)
            x_nat = const.tile([P, M_TILES, D], FP32)
            xbf_sb = const.tile([P, M_TILES, D], BF16)
            xT = [const.tile([P, N_SHARD], BF16, name=f"xT{c}") for c in range(2)]
            yT = [const.tile([P, M_Y], BF16, name=f"yT{c}") for c in range(2)]
            yTsq = const.tile([P, M_Y], BF16, name="yTsq")

            xbf = [dramp.tile([N_SHARD, P], BF16, name=f"xbf{c}") for c in range(2)]
            ybf = [dramp.tile([M_Y, P], BF16, name=f"ybf{c}") for c in range(2)]

            # ---- x: load fp32, row norms, cast*(-2) -> DRAM -> XBAR ----
            nc.sync.dma_start(x_nat[:], x_d.rearrange("(t p) d -> p t d", p=P))
            nc.vector.tensor_scalar_mul(xbf_sb[:], x_nat[:], -2.0)
            for c in range(2):
                nc.sync.dma_start(
                    xbf[c].rearrange("(t p) d -> p t d", p=P),
                    xbf_sb[:, :, c * P : (c + 1) * P],
                )
            for c in range(2):
                nc.scalar.dma_start_transpose(xT[c][:, :], xbf[c][:, :])
            for t in range(M_TILES):
                sq = sqp.tile([P, D], FP32, tag="sq")
                nc.scalar.activation(
                    sq[:], x_nat[:, t, :], AF.Square, accum_out=xsq[:, t : t + 1]
                )

            # ---- y chunk staging: load fp32, cast bf16 ----
            ycast = {}

            def y_load(ch, cast_eng):
                rows = slice(ch * YCH, (ch + 1) * YCH)
                yst = ystage.tile([P, YCH // P, D], FP32, tag="yst")
                nc.sync.dma_start(
                    yst[:], y_d[rows, :].rearrange("(t p) d -> p t d", p=P)
                )
                ybf_sb = ystage.tile([P, YCH // P, D], BF16, tag="ybf_sb")
                cast_eng.tensor_copy(ybf_sb[:], yst[:])
                ycast[ch] = ybf_sb

            def y_store(ch):
                # bounce path for XBAR quarters
                rows = slice(ch * YCH, (ch + 1) * YCH)
                for c in range(2):
                    nc.sync.dma_start(
                        ybf[c][rows, :].rearrange("(t p) d -> p t d", p=P),
                        ycast[ch][:, :, c * P : (c + 1) * P],
                    )

            def y_xbar(q, eng):
                rows = slice(q * QCOLS, (q + 1) * QCOLS)
                for c in range(2):
                    eng.dma_start_transpose(yT[c][:, rows], ybf[c][rows, :])

            def y_petr_chunk(ch):
                # PE transpose one 1024-row chunk of cast bf16 into yT
                src = ycast[ch]
                for c in range(2):
                    ps = psmm.tile([P, YCH], BF16, tag="mm", name=f"tr_{ch}_{c}")
                    for t in range(YCH // P):
                        nc.tensor.transpose(
                            ps[:, t * P : (t + 1) * P],
                            src[:, t, c * P : (c + 1) * P],
                            identity,
                        )
                    cols = slice(ch * YCH, (ch + 1) * YCH)
                    if c == 0:
                        nc.vector.tensor_copy(yT[c][:, cols], ps[:])
                    else:
                        nc.scalar.copy(yT[c][:, cols], ps[:])

            def ytsq_quarter(q):
                cs = slice(q * QCOLS, (q + 1) * QCOLS)
                t0 = sqp.tile([P, QCOLS], BF16, tag="t0")
                nc.vector.tensor_tensor(t0[:], yT[0][:, cs], yT[0][:, cs], ALU.mult)
                t1 = sqp.tile([P, QCOLS], BF16, tag="t1")
                nc.vector.tensor_tensor(t1[:], yT[1][:, cs], yT[1][:, cs], ALU.mult)
                nc.vector.tensor_tensor(yTsq[:, cs], t0[:], t1[:], ALU.add)

            # head: stage q0 (PE transpose) + q1 loads; bounce q2-3 async
            y_load(0, nc.vector)
            y_load(1, nc.vector)
            y_petr_chunk(0)
            y_petr_chunk(1)
            ytsq_quarter(0)
            y_load(2, nc.vector)
            y_load(3, nc.vector)
            y_load(4, nc.gpsimd)
            y_store(4)
            y_load(5, nc.gpsimd)
            y_store(5)

            # ---- main: per (quarter, m-tile): 4 banks x 3 passes ----
            for q in range(Q):
                for m in range(M_TILES):
                    # stream the rest of y behind the main loop
                    if q == 0 and m in (0, 1):
                        y_petr_chunk(2 + m)  # q1 chunks on PE
                    if q == 0 and m == 2:
                        y_load(6, nc.gpsimd)
                        y_store(6)
                    if q == 0 and m == 3:
                        y_load(7, nc.gpsimd)
                        y_store(7)
                    if q == 0 and m == 4:
                        y_xbar(2, nc.sync)
                    if q == 0 and m == 5:
                        y_xbar(3, nc.scalar)
                    if q + 2 < Q and m == 6:
                        ytsq_quarter(q + 1)
                    if q == 2 and m == 2:
                        ytsq_quarter(3)
                    lhs0 = xT[0][:, m * P : (m + 1) * P]
                    lhs1 = xT[1][:, m * P : (m + 1) * P]
                    ot = outp.tile([P, QCOLS], FP16, tag="ot")
                    pms = [
                        psmm.tile([P, NT], FP32, tag="mm", name=f"pm_{q}_{m}_{k}")
                        for k in range(GRP)
                    ]
                    for k in range(GRP):
                        n = q * GRP + k
                        nc.tensor.matmul(
                            pms[k][:], lhs0, yT[0][:, n * NT : (n + 1) * NT],
                            start=True, stop=False,
                        )
                    for k in range(GRP):
                        n = q * GRP + k
                        nc.tensor.matmul(
                            pms[k][:], lhs1, yT[1][:, n * NT : (n + 1) * NT],
                            start=False, stop=False,
                        )
                    for k in range(GRP):
                        n = q * GRP + k
                        nc.tensor.matmul(
                            pms[k][:], ones[:], yTsq[:, n * NT : (n + 1) * NT],
                            start=False, stop=True,
                        )
                    for k in range(GRP):
                        osl = ot[:, k * NT : (k + 1) * NT]
                        if k % 2 == 0:
                            nc.scalar.activation(
                                osl, pms[k][:], AF.Identity,
                                bias=xsq[:, m : m + 1], scale=1.0,
                            )
                        else:
                            nc.vector.tensor_scalar_add(
                                osl, pms[k][:], xsq[:, m : m + 1]
                            )
                    out_eng = nc.sync if m % 2 == 0 else nc.scalar
                    out_eng.dma_start(
                        out_d[m * P : (m + 1) * P, q * QCOLS : (q + 1) * QCOLS],
                        ot[:],
                    )

    nc.compile()
    return nc


def _get_nc():
    if "nc" not in _CACHE:
        _CACHE["nc"] = _build()
    return _CACHE["nc"]


def kernel(x: np.ndarray, y: np.ndarray) -> np.ndarray:
    global LAST_RESULTS
    x = np.ascontiguousarray(np.asarray(x, dtype=np.float32))
    y = np.ascontiguousarray(np.asarray(y, dtype=np.float32))
    assert x.shape == (N_FULL, D) and y.shape == (M_Y, D)

    nc = _get_nc()
    in_maps = [
        {"x": x[i * N_SHARD : (i + 1) * N_SHARD], "y": y} for i in range(N_CORES)
    ]
    res = run_bass_kernel_spmd(
        nc,
        in_maps,
        core_ids=list(range(N_CORES)),
        trace=bool(os.environ.get("BASS_KERNEL_TRACE")),
    )
    LAST_RESULTS = res
    return np.concatenate(
        [res.results[i]["out"].astype(np.float32) for i in range(N_CORES)], axis=0
    )


# revision 9
# speedup vs baseline: 1.4352x; 1.0268x over previous
"""Pairwise squared-euclidean distance kernel for Trainium2 (8 NeuronCores).

z[i, j] = ||x_i||^2 + ||y_j||^2 - 2 * <x_i, y_j>

Sharding: x rows split across 8 cores (1024 rows each), y replicated.
Each core computes a [1024, 8192] tile of the output with no communication.

Per-core algorithm (v5 — hybrid transpose, fp16 out, HAM-warm):
  1. x: fp32 load for row norms (ScalarE Square+accum); DVE casts
     x*(-2) to bf16, bounced through DRAM and XBAR-DMA-transposed back
     as xT [d, 1024].
  2. y quarters 0-1: fp32 chunk loads -> DVE cast bf16 -> PE transpose
     (bf16, 1 cyc/col) -> PSUM -> evac to yT. Fills the otherwise-idle
     PE during the pipeline head.
     y quarters 2-3: DVE-cast chunks bounce through DRAM (contiguous
     d-halves) and return via XBAR DMA-transpose - zero PE cost, using
     later-phase DMA slack. (fp32 XBAR is not supported by HW: wedges
     the device. SWDGE cast-DMA races its consumers - avoid.)
  3. yTsq = yT0^2 + yT1^2 (DVE, bf16): ysq rides the PE as a third
     matmul pass per bank (all-ones stationary broadcasts ysq_j).
  4. Main loop: 32 groups (y quarter x m-tile) of 4 PSUM banks; per
     bank 3 passes (xT0, xT1, ones). Evac = one fused op per bank:
     ScalarE activation(psum + xsq) / DVE tensor_scalar(psum + xsq),
     alternating, straight to fp16. Host upcasts fp16 -> fp32.
  5. Output DMA alternates sync/scalar HWDGE queues; back-to-back PE
     groups keep the HAM clock gate warm (2.4 GHz).

Known-good environment notes: tensor_tensor_reduce crashes the device
(NRT_EXEC_UNIT_UNRECOVERABLE) - do not use. fp32r matmuls never warm
the HAM clock gate and self-load weights serially (~536ns/mm).
"""

import os

import numpy as np

import concourse.bacc as bacc
import concourse.mybir as mybir
import concourse.tile as tile
from concourse.bass_utils import run_bass_kernel_spmd
from concourse.masks import make_identity

N_CORES = 8
N_FULL = 8192  # total x rows
M_Y = 8192  # y rows
D = 256  # feature dim
N_SHARD = N_FULL // N_CORES  # 1024 x rows per core

P = 128
NT = 512  # matmul free-dim tile (one fp32 PSUM bank)
GRP = 4  # PSUM banks per group (one y quarter)
QCOLS = GRP * NT  # 2048
Q = M_Y // QCOLS  # 4 y quarters
M_TILES = N_SHARD // P  # 8
YCH = 1024  # y rows per load chunk
PE_Q = 2  # quarters 0..PE_Q-1 transposed on the PE; rest via XBAR

FP32 = mybir.dt.float32
BF16 = mybir.dt.bfloat16
FP16 = mybir.dt.float16
AF = mybir.ActivationFunctionType
ALU = mybir.AluOpType

_CACHE = {}
LAST_RESULTS = None


def _build():
    nc = bacc.Bacc("TRN2", target_bir_lowering=False, debug=False, num_devices=N_CORES)
    x_d = nc.dram_tensor("x", [N_SHARD, D], FP32, kind="ExternalInput").ap()
    y_d = nc.dram_tensor("y", [M_Y, D], FP32, kind="ExternalInput").ap()
    out_d = nc.dram_tensor("out", [N_SHARD, M_Y], FP16, kind="ExternalOutput").ap()

    with tile.TileContext(nc) as tc:
        with (
            tc.tile_pool(name="const", bufs=1) as const,
            tc.tile_pool(name="sq", bufs=4) as sqp,
            tc.tile_pool(name="ystage", bufs=4) as ystage,
            tc.tile_pool(name="outp", bufs=3) as outp,
            tc.tile_pool(name="dramp", bufs=1, space="DRAM") as dramp,
            tc.tile_pool(name="psmm", bufs=8, space="PSUM") as psmm,
        ):
            ones = const.tile([P, P], BF16)
            nc.gpsimd.memset(ones[:], 1.0)
            identity = const.tile([P, P], BF16)
            make_identity(nc, identity)

            xsq = const.tile([P, M_TILES], FP32)
            x_nat = const.tile([P, M_TILES, D], FP32)
            xbf_sb = const.tile([P, M_TILES, D], BF16)
            xT = [const.tile([P, N_SHARD], BF16, name=f"xT{c}") for c in range(2)]
            yT = [const.tile([P, M_Y], BF16, name=f"yT{c}") for c in range(2)]
            yTsq = const.tile([P, M_Y], BF16, name="yTsq")

            xbf = [dramp.tile([N_SHARD, P], BF16, name=f"xbf{c}") for c in range(2)]
            ybf = [dramp.tile([M_Y, P], BF16, name=f"ybf{c}") for c in range(2)]

            # ---- x: load fp32, row norms, cast*(-2) -> DRAM -> XBAR ----
            nc.sync.dma_start(x_nat[:], x_d.rearrange("(t p) d -> p t d", p=P))
            nc.vector.tensor_scalar_mul(xbf_sb[:], x_nat[:], -2.0)
            for c in range(2):
                nc.sync.dma_start(
                    xbf[c].rearrange("(t p) d -> p t d", p=P),
                    xbf_sb[:, :, c * P : (c + 1) * P],
                )
            for c in range(2):
                nc.scalar.dma_start_transpose(xT[c][:, :], xbf[c][:, :])
            for t in range(M_TILES):
                sq = sqp.tile([P, D], FP32, tag="sq")
                nc.scalar.activation(
                    sq[:], x_nat[:, t, :], AF.Square, accum_out=xsq[:, t : t + 1]
                )

            # ---- y chunk staging: load fp32, cast bf16 ----
            ycast = {}

            def y_load(ch, cast_eng):
                rows = slice(ch * YCH, (ch + 1) * YCH)
                yst = ystage.tile([P, YCH // P, D], FP32, tag="yst")
                nc.sync.dma_start(
                    yst[:], y_d[rows, :].rearrange("(t p) d -> p t d", p=P)
                )
                ybf_sb = ystage.tile([P, YCH // P, D], BF16, tag="ybf_sb")
                if cast_eng is nc.scalar:
                    cast_eng.copy(ybf_sb[:], yst[:])
                else:
                    cast_eng.tensor_copy(ybf_sb[:], yst[:])
                ycast[ch] = ybf_sb

            def y_store(ch):
                # bounce path for XBAR quarters
                rows = slice(ch * YCH, (ch + 1) * YCH)
                for c in range(2):
                    nc.sync.dma_start(
                        ybf[c][rows, :].rearrange("(t p) d -> p t d", p=P),
                        ycast[ch][:, :, c * P : (c + 1) * P],
                    )

            def y_xbar(q, eng):
                rows = slice(q * QCOLS, (q + 1) * QCOLS)
                for c in range(2):
                    eng.dma_start_transpose(yT[c][:, rows], ybf[c][rows, :])

            def y_petr_chunk(ch):
                # PE transpose one 1024-row chunk of cast bf16 into yT
                src = ycast[ch]
                for c in range(2):
                    ps = psmm.tile([P, YCH], BF16, tag="mm", name=f"tr_{ch}_{c}")
                    for t in range(YCH // P):
                        nc.tensor.transpose(
                            ps[:, t * P : (t + 1) * P],
                            src[:, t, c * P : (c + 1) * P],
                            identity,
                        )
                    cols = slice(ch * YCH, (ch + 1) * YCH)
                    if c == 0:
                        nc.vector.tensor_copy(yT[c][:, cols], ps[:])
                    else:
                        nc.scalar.copy(yT[c][:, cols], ps[:])

            def ytsq_chunk(ch):
                cs = slice(ch * YCH, (ch + 1) * YCH)
                t0 = sqp.tile([P, YCH], BF16, tag="t0")
                nc.vector.tensor_tensor(t0[:], yT[0][:, cs], yT[0][:, cs], ALU.mult)
                t1 = sqp.tile([P, YCH], BF16, tag="t1")
                nc.vector.tensor_tensor(t1[:], yT[1][:, cs], yT[1][:, cs], ALU.mult)
                nc.vector.tensor_tensor(yTsq[:, cs], t0[:], t1[:], ALU.add)

            # head: stage q0 (PE transpose) + q1 loads; bounce q2-3 async
            y_load(0, nc.vector)
            y_load(1, nc.vector)
            y_petr_chunk(0)
            ytsq_chunk(0)
            y_petr_chunk(1)
            ytsq_chunk(1)
            y_load(4, nc.gpsimd)
            y_store(4)
            y_load(5, nc.gpsimd)
            y_store(5)

            # ---- main: per (quarter, m-tile): 4 banks x 3 passes ----
            for q in range(Q):
                for m in range(M_TILES):
                    # stream the rest of y behind the main loop
                    if q == 0 and m in (0, 1):
                        y_load(2 + m, nc.vector)
                        y_petr_chunk(2 + m)  # q1 chunks on PE
                        ytsq_chunk(2 + m)
                    if q == 0 and m == 2:
                        y_load(6, nc.scalar)
                        y_store(6)
                    if q == 0 and m == 3:
                        y_load(7, nc.scalar)
                        y_store(7)
                    if q == 0 and m == 4:
                        y_xbar(2, nc.sync)
                    if q == 1 and m == 1:
                        y_xbar(3, nc.sync)
                    if q == 1 and m in (2, 3):
                        ytsq_chunk(2 + m)  # chunks 4,5 (q2)
                    if q == 2 and m in (1, 2):
                        ytsq_chunk(5 + m)  # chunks 6,7 (q3)
                    lhs0 = xT[0][:, m * P : (m + 1) * P]
                    lhs1 = xT[1][:, m * P : (m + 1) * P]
                    ot = outp.tile([P, QCOLS], FP16, tag="ot")
                    pms = [
                        psmm.tile([P, NT], FP32, tag="mm", name=f"pm_{q}_{m}_{k}")
                        for k in range(GRP)
                    ]
                    for k in range(GRP):
                        n = q * GRP + k
                        nc.tensor.matmul(
                            pms[k][:], lhs0, yT[0][:, n * NT : (n + 1) * NT],
                            start=True, stop=False,
                        )
                    for k in range(GRP):
                        n = q * GRP + k
                        nc.tensor.matmul(
                            pms[k][:], lhs1, yT[1][:, n * NT : (n + 1) * NT],
                            start=False, stop=False,
                        )
                    for k in range(GRP):
                        n = q * GRP + k
                        nc.tensor.matmul(
                            pms[k][:], ones[:], yTsq[:, n * NT : (n + 1) * NT],
                            start=False, stop=True,
                        )
                    for k in range(GRP):
                        osl = ot[:, k * NT : (k + 1) * NT]
                        if k % 2 == 0:
                            nc.scalar.activation(
                                osl, pms[k][:], AF.Identity,
                                bias=xsq[:, m : m + 1], scale=1.0,
                            )
                        else:
                            nc.vector.tensor_scalar_add(
                                osl, pms[k][:], xsq[:, m : m + 1]
                            )
                    out_eng = nc.sync if m % 2 == 0 else nc.scalar
                    out_eng.dma_start(
                        out_d[m * P : (m + 1) * P, q * QCOLS : (q + 1) * QCOLS],
                        ot[:],
                    )

    nc.compile()
    return nc


def _get_nc():
    if "nc" not in _CACHE:
        _CACHE["nc"] = _build()
    return _CACHE["nc"]


def kernel(x: np.ndarray, y: np.ndarray) -> np.ndarray:
    global LAST_RESULTS
    x = np.ascontiguousarray(np.asarray(x, dtype=np.float32))
    y = np.ascontiguousarray(np.asarray(y, dtype=np.float32))
    assert x.shape == (N_FULL, D) and y.shape == (M_Y, D)

    nc = _get_nc()
    in_maps = [
        {"x": x[i * N_SHARD : (i + 1) * N_SHARD], "y": y} for i in range(N_CORES)
    ]
    res = run_bass_kernel_spmd(
        nc,
        in_maps,
        core_ids=list(range(N_CORES)),
        trace=bool(os.environ.get("BASS_KERNEL_TRACE")),
    )
    LAST_RESULTS = res
    return np.concatenate(
        [res.results[i]["out"].astype(np.float32) for i in range(N_CORES)], axis=0
    )


# revision 10
# speedup vs baseline: 1.5932x; 1.1101x over previous
"""Pairwise squared-euclidean distance kernel for Trainium2 (8 NeuronCores).

z[i, j] = ||x_i||^2 + ||y_j||^2 - 2 * <x_i, y_j>

Sharding: x rows split across 8 cores (1024 rows each), y replicated.
Each core computes a [1024, 8192] tile of the output with no communication.

Per-core algorithm (v6 — hybrid transpose, fp16 out, HAM-warm):
  1. x: fp32 load for row norms (ScalarE Square+accum); DVE casts
     x*(-2) to bf16; PE-transposed (16 tiles) during the pipeline head.
  2. y quarters 0-1: fp32 chunk loads -> DVE cast bf16 -> PE transpose
     (bf16, 1 cyc/col) -> PSUM -> evac to yT, interleaved with the
     first main groups so the PE never idles.
     y quarters 2-3: ScalarE-cast chunks bounce through DRAM
     (contiguous d-halves) and return via XBAR DMA-transpose - zero PE
     cost, using later-phase DMA slack. (fp32 XBAR is not supported by
     HW: wedges the device. SWDGE cast-DMA races its consumers, and
     gpsimd engine casts are 6x slower than DVE/ACT - avoid all.)
  3. yTsq = yT0^2 + yT1^2 (DVE, bf16, per chunk): ysq rides the PE as
     a third matmul pass per bank (all-ones stationary broadcasts
     ysq_j to every partition).
  4. Main loop: 32 groups (y quarter x m-tile) of 4 PSUM banks; per
     bank 3 passes (xT0, xT1, ones). Evac = one fused op per bank:
     ScalarE activation(psum + xsq) / DVE tensor_scalar(psum + xsq),
     alternating, straight to fp16. Host upcasts fp16 -> fp32.
  5. Output DMA alternates sync/scalar HWDGE queues; back-to-back PE
     groups keep the HAM clock gate warm (2.4 GHz).

Known-good environment notes: tensor_tensor_reduce crashes the device
(NRT_EXEC_UNIT_UNRECOVERABLE) - do not use. fp32r matmuls never warm
the HAM clock gate and self-load weights serially (~536ns/mm).
"""

import os

import numpy as np

import concourse.bacc as bacc
import concourse.mybir as mybir
import concourse.tile as tile
from concourse.bass_utils import run_bass_kernel_spmd
from concourse.masks import make_identity

N_CORES = 8
N_FULL = 8192  # total x rows
M_Y = 8192  # y rows
D = 256  # feature dim
N_SHARD = N_FULL // N_CORES  # 1024 x rows per core

P = 128
NT = 512  # matmul free-dim tile (one fp32 PSUM bank)
GRP = 4  # PSUM banks per group (one y quarter)
QCOLS = GRP * NT  # 2048
Q = M_Y // QCOLS  # 4 y quarters
M_TILES = N_SHARD // P  # 8
YCH = 1024  # y rows per load chunk

FP32 = mybir.dt.float32
BF16 = mybir.dt.bfloat16
FP16 = mybir.dt.float16
AF = mybir.ActivationFunctionType
ALU = mybir.AluOpType

_CACHE = {}
LAST_RESULTS = None


def _build():
    nc = bacc.Bacc("TRN2", target_bir_lowering=False, debug=False, num_devices=N_CORES)
    x_d = nc.dram_tensor("x", [N_SHARD, D], FP32, kind="ExternalInput").ap()
    y_d = nc.dram_tensor("y", [M_Y, D], FP32, kind="ExternalInput").ap()
    out_d = nc.dram_tensor("out", [N_SHARD, M_Y], FP16, kind="ExternalOutput").ap()

    with tile.TileContext(nc) as tc:
        with (
            tc.tile_pool(name="const", bufs=1) as const,
            tc.tile_pool(name="sq", bufs=4) as sqp,
            tc.tile_pool(name="ystage", bufs=5) as ystage,
            tc.tile_pool(name="outp", bufs=3) as outp,
            tc.tile_pool(name="dramp", bufs=1, space="DRAM") as dramp,
            tc.tile_pool(name="psmm", bufs=8, space="PSUM") as psmm,
        ):
            ones = const.tile([P, P], BF16)
            nc.vector.memset(ones[:], 1.0)
            identity = const.tile([P, P], BF16)
            make_identity(nc, identity)

            xsq = const.tile([P, M_TILES], FP32)
            x_nat = const.tile([P, M_TILES, D], FP32)
            xbf_sb = const.tile([P, M_TILES, D], BF16)
            xT = [const.tile([P, N_SHARD], BF16, name=f"xT{c}") for c in range(2)]
            yT = [const.tile([P, M_Y], BF16, name=f"yT{c}") for c in range(2)]
            yTsq = const.tile([P, M_Y], BF16, name="yTsq")

            ybf = [dramp.tile([M_Y, P], BF16, name=f"ybf{c}") for c in range(2)]

            # ---- x: load fp32, cast*(-2), PE transpose, row norms ----
            nc.sync.dma_start(x_nat[:], x_d.rearrange("(t p) d -> p t d", p=P))
            nc.vector.tensor_scalar_mul(xbf_sb[:], x_nat[:], -2.0)
            for c in range(2):
                ps = psmm.tile([P, N_SHARD], BF16, tag="mm", name=f"xtr{c}")
                for t in range(M_TILES):
                    nc.tensor.transpose(
                        ps[:, t * P : (t + 1) * P],
                        xbf_sb[:, t, c * P : (c + 1) * P],
                        identity,
                    )
                if c == 0:
                    nc.vector.tensor_copy(xT[c][:, :], ps[:])
                else:
                    nc.scalar.copy(xT[c][:, :], ps[:])
            for t in range(M_TILES):
                sq = sqp.tile([P, D], FP32, tag="sq")
                nc.scalar.activation(
                    sq[:], x_nat[:, t, :], AF.Square, accum_out=xsq[:, t : t + 1]
                )

            # ---- y chunk staging ----
            ycast = {}

            def y_load(ch):
                rows = slice(ch * YCH, (ch + 1) * YCH)
                yst = ystage.tile([P, YCH // P, D], FP32, tag="yst")
                nc.sync.dma_start(
                    yst[:], y_d[rows, :].rearrange("(t p) d -> p t d", p=P)
                )
                ycast[ch] = yst

            def y_cast(ch, eng):
                ybf_sb = ystage.tile([P, YCH // P, D], BF16, tag="ybf_sb")
                if eng is nc.scalar:
                    eng.copy(ybf_sb[:], ycast[ch][:])
                else:
                    eng.tensor_copy(ybf_sb[:], ycast[ch][:])
                ycast[ch] = ybf_sb

            def y_store(ch):
                rows = slice(ch * YCH, (ch + 1) * YCH)
                for c in range(2):
                    nc.sync.dma_start(
                        ybf[c][rows, :].rearrange("(t p) d -> p t d", p=P),
                        ycast[ch][:, :, c * P : (c + 1) * P],
                    )

            def y_xbar(q):
                rows = slice(q * QCOLS, (q + 1) * QCOLS)
                for c in range(2):
                    nc.sync.dma_start_transpose(yT[c][:, rows], ybf[c][rows, :])

            def y_petr_chunk(ch):
                src = ycast[ch]
                for c in range(2):
                    ps = psmm.tile([P, YCH], BF16, tag="mm", name=f"tr_{ch}_{c}")
                    for t in range(YCH // P):
                        nc.tensor.transpose(
                            ps[:, t * P : (t + 1) * P],
                            src[:, t, c * P : (c + 1) * P],
                            identity,
                        )
                    cols = slice(ch * YCH, (ch + 1) * YCH)
                    if c == 0:
                        nc.vector.tensor_copy(yT[c][:, cols], ps[:])
                    else:
                        nc.scalar.copy(yT[c][:, cols], ps[:])

            def ytsq_chunk(ch):
                cs = slice(ch * YCH, (ch + 1) * YCH)
                t0 = sqp.tile([P, YCH], BF16, tag="t0")
                nc.vector.tensor_tensor(t0[:], yT[0][:, cs], yT[0][:, cs], ALU.mult)
                t1 = sqp.tile([P, YCH], BF16, tag="t1")
                nc.vector.tensor_tensor(t1[:], yT[1][:, cs], yT[1][:, cs], ALU.mult)
                nc.vector.tensor_tensor(yTsq[:, cs], t0[:], t1[:], ALU.add)

            # head: q0+q1 chunks load first; q0 PE-transposed immediately
            for ch in range(4):
                y_load(ch)
            y_cast(0, nc.vector)
            y_petr_chunk(0)
            ytsq_chunk(0)
            y_cast(1, nc.vector)
            y_petr_chunk(1)
            ytsq_chunk(1)

            # ---- main: per (quarter, m-tile): 4 banks x 3 passes ----
            for q in range(Q):
                for m in range(M_TILES):
                    # stream the rest of y behind the main loop
                    if q == 0 and m in (0, 1):
                        y_cast(2 + m, nc.vector)
                        y_petr_chunk(2 + m)  # q1 chunks on PE
                        ytsq_chunk(2 + m)
                    if q == 0 and m in (2, 3, 4, 5):
                        ch = 2 + m
                        y_load(ch)
                        y_cast(ch, nc.scalar)
                        y_store(ch)
                    if q == 0 and m == 6:
                        y_xbar(2)
                    if q == 0 and m == 7:
                        y_xbar(3)
                    if q == 1 and m in (2, 3):
                        ytsq_chunk(2 + m)  # chunks 4,5 (q2)
                    if q == 2 and m in (1, 2):
                        ytsq_chunk(5 + m)  # chunks 6,7 (q3)
                    lhs0 = xT[0][:, m * P : (m + 1) * P]
                    lhs1 = xT[1][:, m * P : (m + 1) * P]
                    ot = outp.tile([P, QCOLS], FP16, tag="ot")
                    pms = [
                        psmm.tile([P, NT], FP32, tag="mm", name=f"pm_{q}_{m}_{k}")
                        for k in range(GRP)
                    ]
                    for k in range(GRP):
                        n = q * GRP + k
                        nc.tensor.matmul(
                            pms[k][:], lhs0, yT[0][:, n * NT : (n + 1) * NT],
                            start=True, stop=False,
                        )
                    for k in range(GRP):
                        n = q * GRP + k
                        nc.tensor.matmul(
                            pms[k][:], lhs1, yT[1][:, n * NT : (n + 1) * NT],
                            start=False, stop=False,
                        )
                    for k in range(GRP):
                        n = q * GRP + k
                        nc.tensor.matmul(
                            pms[k][:], ones[:], yTsq[:, n * NT : (n + 1) * NT],
                            start=False, stop=True,
                        )
                    for k in range(GRP):
                        osl = ot[:, k * NT : (k + 1) * NT]
                        if k % 2 == 0:
                            nc.scalar.activation(
                                osl, pms[k][:], AF.Identity,
                                bias=xsq[:, m : m + 1], scale=1.0,
                            )
                        else:
                            nc.vector.tensor_scalar_add(
                                osl, pms[k][:], xsq[:, m : m + 1]
                            )
                    out_eng = nc.sync if m % 2 == 0 else nc.scalar
                    out_eng.dma_start(
                        out_d[m * P : (m + 1) * P, q * QCOLS : (q + 1) * QCOLS],
                        ot[:],
                    )

    nc.compile()
    return nc


def _get_nc():
    if "nc" not in _CACHE:
        _CACHE["nc"] = _build()
    return _CACHE["nc"]


def kernel(x: np.ndarray, y: np.ndarray) -> np.ndarray:
    global LAST_RESULTS
    x = np.ascontiguousarray(np.asarray(x, dtype=np.float32))
    y = np.ascontiguousarray(np.asarray(y, dtype=np.float32))
    assert x.shape == (N_FULL, D) and y.shape == (M_Y, D)

    nc = _get_nc()
    in_maps = [
        {"x": x[i * N_SHARD : (i + 1) * N_SHARD], "y": y} for i in range(N_CORES)
    ]
    res = run_bass_kernel_spmd(
        nc,
        in_maps,
        core_ids=list(range(N_CORES)),
        trace=bool(os.environ.get("BASS_KERNEL_TRACE")),
    )
    LAST_RESULTS = res
    return np.concatenate(
        [res.results[i]["out"].astype(np.float32) for i in range(N_CORES)], axis=0
    )
